# revision 6
# baseline (speedup 1.0000x reference)
"""CrossAttentionBlockLLaMA on 8 Trainium2 NeuronCores (Bass/Tile).

Sharding:
  - QKV + attention: tensor-parallel over heads (2 heads/core).
  - Output projection wo: row-sharded over heads; each core computes a
    partial h for ALL tokens, written window-major [8, D, TC]; a
    ReduceScatter sums partials and hands core r exactly h.T[:, tokens_r].
  - FFN + post-norm: token-parallel (TC tokens/core), full weights.

Layouts: host pre-transposes activations/weights so every matmul's
contraction dim is on SBUF partitions. attn_norm_w and 1/sqrt(HD) are
folded into wq/wk/wv host-side; per-token 1/rms factors are applied to
q/k/v on device. Matmul inputs fp16 (validated ~6e-7 end-to-end rel err),
PSUM accumulation fp32, residual + final norm fp32.

Self-contained: hardcodes shapes from the problem spec.
"""
import numpy as np

NCORES = 8
EPS = 1e-5


class Cfg:
    def __init__(self, B=2, S=2048, D=2048, H=16, HD=128, FF=5632):
        self.B, self.S, self.D, self.H, self.HD, self.FF = B, S, D, H, HD, FF
        self.T = B * S                    # total tokens
        self.TC = self.T // NCORES        # tokens per core (phase 3)
        self.NQ = (H // NCORES) * HD      # per-core head dims
        self.DT = D // 128                # d-tiles
        self.FT = FF // 128               # ff-tiles
        self.NQT = self.NQ // 128         # per-core head-dim tiles
        self.TCH = min(512, self.T)       # phase-1 token chunk
        self.QCH = min(512, S)            # phase-2 query chunk
        self.TCW = min(512, self.TC)      # phase-3 / wo token chunk
        assert self.T % self.TCH == 0 and S % self.QCH == 0
        assert self.TC % self.TCW == 0 and S % 128 == 0
        assert HD == 128 and D % 128 == 0 and FF % 128 == 0


FULL = Cfg()


def build(cfg=FULL):
    import concourse.mybir as mybir
    import concourse.tile as tile
    from concourse import bacc

    F16 = mybir.dt.float16
    F32 = mybir.dt.float32

    c = cfg
    nc = bacc.Bacc("TRN2", target_bir_lowering=False, debug=False,
                   num_devices=NCORES)

    ins = {}
    outs = {}
    for s in ("x", "y"):
        ins[f"{s}T"] = nc.dram_tensor(f"{s}T", [c.D, c.T], F16,
                                      kind="ExternalInput").ap()
        for w in ("wq", "wk", "wv"):
            ins[f"{w}T_{s}"] = nc.dram_tensor(
                f"{w}T_{s}", [c.D, c.NQ], F16, kind="ExternalInput").ap()
        ins[f"woT_{s}"] = nc.dram_tensor(
            f"woT_{s}", [c.NQ, c.D], F16, kind="ExternalInput").ap()
        ins[f"w1T_{s}"] = nc.dram_tensor(
            f"w1T_{s}", [c.D, c.FF], F16, kind="ExternalInput").ap()
        ins[f"w3T_{s}"] = nc.dram_tensor(
            f"w3T_{s}", [c.D, c.FF], F16, kind="ExternalInput").ap()
        ins[f"w2T_{s}"] = nc.dram_tensor(
            f"w2T_{s}", [c.FF, c.D], F16, kind="ExternalInput").ap()
        ins[f"res_{s}"] = nc.dram_tensor(
            f"res_{s}", [c.D, c.TC], F32, kind="ExternalInput").ap()
        ins[f"fnorm_{s}"] = nc.dram_tensor(
            f"fnorm_{s}", [128, c.DT], F32, kind="ExternalInput").ap()
        outs[s] = nc.dram_tensor(f"out_{s}", [c.D, c.TC], F32,
                                 kind="ExternalOutput").ap()

    with tile.TileContext(nc) as tc:
        _emit(tc, nc, c, ins, outs)
    nc.compile()
    return nc


def _emit(tc, nc, c, ins, outs):
    import concourse.mybir as mybir

    F16 = mybir.dt.float16
    F32 = mybir.dt.float32
    AF = mybir.ActivationFunctionType
    one_over_d = 1.0 / c.D

    with (
        tc.tile_pool(name="psum", bufs=1, space="PSUM") as ps,
        tc.tile_pool(name="const", bufs=1) as const,
        tc.tile_pool(name="dram", bufs=1, space="DRAM") as dram,
    ):
        ones_col = const.tile([128, 1], F16)
        nc.vector.memset(ones_col[:], 1.0)
        ones_row = const.tile([1, 128], F16)
        nc.vector.memset(ones_row[:], 1.0)
        one11 = const.tile([1, 1], F32)
        nc.vector.memset(one11[:], 1.0)
        eps1 = const.tile([1, 1], F32)
        nc.vector.memset(eps1[:], EPS)

        sc = {}
        for s in ("x", "y"):
            sc[f"qT_{s}"] = dram.tile([c.NQ, c.T], F16, name=f"qT_{s}")
            sc[f"kT_{s}"] = dram.tile([c.NQ, c.T], F16, name=f"kT_{s}")
            sc[f"v_{s}"] = dram.tile([c.T, c.NQ], F16, name=f"v_{s}")
            sc[f"o_{s}"] = dram.tile([c.NQ, c.T], F16, name=f"o_{s}")
            # wo partials, window-major: [NCORES windows, D, TC]
            sc[f"hp_{s}"] = dram.tile([NCORES * c.D, c.TC], F16,
                                      name=f"hp_{s}")
            sc[f"h_{s}"] = dram.tile([c.D, c.TC], F16, name=f"h_{s}")

        def mm(shape, name):
            return ps.tile(shape, F32, tag="mm", bufs=6, name=name)

        def row(shape, name):
            return ps.tile(shape, F32, tag="row", bufs=2, name=name)

        def bcast_free(rsq16, width, sb_pool, name):
            """[1,width] f16 -> [128,width] f16 via K=1 outer-product MM."""
            bc_ps = mm([128, width], f"bcp_{name}")
            nc.tensor.matmul(bc_ps[:], ones_row[:], rsq16[:1, :width],
                             start=True, stop=True)
            bc16 = sb_pool.tile([128, width], F16, tag="bc16",
                                name=f"bc16_{name}")
            nc.scalar.activation(bc16[:], bc_ps[:], AF.Copy)
            return bc16

        # ============ PHASE 1: RMSNorm stats + QKV projections =============
        with (
            tc.tile_pool(name="p1w", bufs=1) as p1w,
            tc.tile_pool(name="p1a", bufs=2) as p1a,
            tc.tile_pool(name="p1s", bufs=3) as p1s,
        ):
            W = {}
            for s in ("x", "y"):
                for w in ("wq", "wk", "wv"):
                    t = p1w.tile([128, c.DT, c.NQ], F16, name=f"{w}_{s}_sb")
                    nc.sync.dma_start(
                        t[:],
                        ins[f"{w}T_{s}"].rearrange("(o p) j -> p o j", p=128))
                    W[f"{w}{s}"] = t

            for ich in range(c.T // c.TCH):
                tsl = slice(ich * c.TCH, (ich + 1) * c.TCH)
                act = {}
                rsq_free = {}
                rsq_part = {}
                for s in ("x", "y"):
                    at = p1a.tile([128, c.DT, c.TCH], F16, tag=f"act_{s}",
                                  name=f"act_{s}")
                    nc.sync.dma_start(
                        at[:],
                        ins[f"{s}T"][:, tsl].rearrange("(o p) t -> p o t",
                                                       p=128))
                    act[s] = at

                    ms_ps = row([1, c.TCH], f"ms_{s}")
                    for o in range(c.DT):
                        sq = p1s.tile([128, c.TCH], F16, tag="sq",
                                      name=f"sq_{s}{o}")
                        nc.vector.tensor_mul(sq[:], at[:, o], at[:, o])
                        nc.tensor.matmul(ms_ps[:], ones_col[:], sq[:],
                                         start=(o == 0), stop=(o == c.DT - 1))
                    rms = p1s.tile([1, c.TCH], F32, tag="rms",
                                   name=f"rms_{s}")
                    nc.scalar.activation(rms[:], ms_ps[:], AF.Sqrt,
                                         bias=eps1[:], scale=one_over_d)
                    rsqf = p1s.tile([1, c.TCH], F32, tag="rsqf",
                                    name=f"rsqf_{s}")
                    nc.vector.reciprocal(rsqf[:], rms[:])
                    rsqf16 = p1s.tile([1, c.TCH], F16, tag="rsqf16",
                                      name=f"rsqf16_{s}")
                    nc.vector.tensor_copy(rsqf16[:], rsqf[:])
                    rsq_free[s] = rsqf16

                    nsub = c.TCH // 128
                    rsqT = p1s.tile([128, nsub], F32, tag="rsqT",
                                    name=f"rsqT_{s}")
                    for i in range(nsub):
                        tp = mm([128, 1], f"tp_{s}{i}")
                        nc.tensor.matmul(
                            tp[:], rsqf[:1, i * 128:(i + 1) * 128], one11[:],
                            start=True, stop=True)
                        nc.vector.tensor_copy(rsqT[:, i:i + 1], tp[:])
                    rsq_part[s] = rsqT

                for s in ("x", "y"):
                    kv = "y" if s == "x" else "x"
                    bc_q = bcast_free(rsq_free[s], c.TCH, p1s, f"q{s}{ich}")
                    bc_k = bcast_free(rsq_free[kv], c.TCH, p1s, f"k{s}{ich}")

                    for (wname, src, bc, dst) in (
                        ("wq", s, bc_q, sc[f"qT_{s}"]),
                        ("wk", kv, bc_k, sc[f"kT_{s}"]),
                    ):
                        for jt in range(c.NQT):
                            pm = mm([128, c.TCH], f"{wname}{s}{jt}")
                            wt = W[f"{wname}{s}"]
                            for o in range(c.DT):
                                nc.tensor.matmul(
                                    pm[:], wt[:, o, jt * 128:(jt + 1) * 128],
                                    act[src][:, o],
                                    start=(o == 0), stop=(o == c.DT - 1))
                            ot = p1s.tile([128, c.TCH], F16, tag="proj_out",
                                          name=f"{wname}{s}{jt}o")
                            nc.vector.tensor_mul(ot[:], pm[:], bc[:])
                            nc.sync.dma_start(
                                dst[jt * 128:(jt + 1) * 128, tsl], ot[:])

                    for i in range(c.TCH // 128):
                        pv = mm([128, c.NQ], f"v{s}{i}")
                        for o in range(c.DT):
                            nc.tensor.matmul(
                                pv[:], act[kv][:, o, i * 128:(i + 1) * 128],
                                W[f"wv{s}"][:, o, :],
                                start=(o == 0), stop=(o == c.DT - 1))
                        vt = p1s.tile([128, c.NQ], F16, tag="v_out",
                                      name=f"v{s}{i}o")
                        nc.vector.tensor_scalar_mul(
                            vt[:], pv[:], rsq_part[kv][:, i:i + 1])
                        nc.sync.dma_start(
                            sc[f"v_{s}"][ich * c.TCH + i * 128:
                                         ich * c.TCH + (i + 1) * 128, :],
                            vt[:])

        # ============ PHASE 2: attention + wo partial + ReduceScatter ======
        for s in ("x", "y"):
            with tc.tile_pool(name=f"p2_{s}", bufs=2) as p2:
                for b in range(c.B):
                    bsl = slice(b * c.S, (b + 1) * c.S)
                    for h in range(c.NQT):
                        hsl = slice(h * 128, (h + 1) * 128)
                        kt = p2.tile([128, c.S], F16, tag="kt", name="kt")
                        nc.sync.dma_start(kt[:], sc[f"kT_{s}"][hsl, bsl])
                        vt = p2.tile([128, c.S // 128, 128], F16, tag="vt",
                                     name="vt")
                        nc.sync.dma_start(
                            vt[:], sc[f"v_{s}"][bsl, hsl].rearrange(
                                "(n p) j -> p n j", p=128))
                        for q0 in range(0, c.S, c.QCH):
                            qsl = slice(b * c.S + q0, b * c.S + q0 + c.QCH)
                            qt = p2.tile([128, c.QCH], F16, tag="qt",
                                         name="qt")
                            nc.sync.dma_start(qt[:], sc[f"qT_{s}"][hsl, qsl])
                            o_ps = mm([128, c.QCH], "o_ps")
                            sum_ps = row([1, c.QCH], "sum_ps")
                            nk = c.S // 128
                            for ik in range(nk):
                                s_ps = mm([128, c.QCH], "s_ps")
                                nc.tensor.matmul(
                                    s_ps[:], kt[:, ik * 128:(ik + 1) * 128],
                                    qt[:], start=True, stop=True)
                                e16 = p2.tile([128, c.QCH], F16, tag="e16",
                                              bufs=4, name="e16")
                                nc.scalar.activation(e16[:], s_ps[:], AF.Exp)
                                nc.tensor.matmul(sum_ps[:], ones_col[:],
                                                 e16[:], start=(ik == 0),
                                                 stop=(ik == nk - 1))
                                nc.tensor.matmul(o_ps[:], vt[:, ik], e16[:],
                                                 start=(ik == 0),
                                                 stop=(ik == nk - 1))
                            rs_ = p2.tile([1, c.QCH], F32, tag="rs",
                                          name="rs")
                            nc.vector.reciprocal(rs_[:], sum_ps[:])
                            rs16 = p2.tile([1, c.QCH], F16, tag="rs16",
                                           name="rs16")
                            nc.vector.tensor_copy(rs16[:], rs_[:])
                            bc16 = bcast_free(rs16, c.QCH, p2, "at")
                            on16 = p2.tile([128, c.QCH], F16, tag="on16",
                                           name="on16")
                            nc.vector.tensor_mul(on16[:], o_ps[:], bc16[:])
                            nc.sync.dma_start(sc[f"o_{s}"][hsl, qsl],
                                              on16[:])

                # ---- wo partial for ALL tokens, written window-major ----
                wo_sb = p2.tile([128, c.NQT, c.D], F16, tag="wo", bufs=1,
                                name="wo_sb")
                nc.sync.dma_start(
                    wo_sb[:],
                    ins[f"woT_{s}"].rearrange("(o p) j -> p o j", p=128))
                for w in range(NCORES):
                    for u in range(c.TC // c.TCW):
                        t0 = w * c.TC + u * c.TCW
                        ot = p2.tile([128, c.NQT, c.TCW], F16, tag="ot",
                                     name="ot")
                        nc.sync.dma_start(
                            ot[:], sc[f"o_{s}"][:, t0:t0 + c.TCW].rearrange(
                                "(o p) t -> p o t", p=128))
                        for dt in range(c.DT):
                            hp = mm([128, c.TCW], "hp")
                            for o in range(c.NQT):
                                nc.tensor.matmul(
                                    hp[:],
                                    wo_sb[:, o, dt * 128:(dt + 1) * 128],
                                    ot[:, o], start=(o == 0),
                                    stop=(o == c.NQT - 1))
                            hp16 = p2.tile([128, c.TCW], F16, tag="hp16",
                                           name="hp16")
                            nc.scalar.activation(hp16[:], hp[:], AF.Copy)
                            nc.sync.dma_start(
                                sc[f"hp_{s}"][w * c.D + dt * 128:
                                              w * c.D + (dt + 1) * 128,
                                              u * c.TCW:(u + 1) * c.TCW],
                                hp16[:])

            nc.gpsimd.collective_compute(
                "ReduceScatter", mybir.AluOpType.add,
                replica_groups=[list(range(NCORES))],
                ins=[sc[f"hp_{s}"][:].opt()],
                outs=[sc[f"h_{s}"][:].opt()],
            )

        # ============ PHASE 3: SwiGLU FFN + residual + post-norm ===========
        for s in ("x", "y"):
            with (
                tc.tile_pool(name=f"p3_{s}", bufs=1) as p3,
                tc.tile_pool(name=f"p3w_{s}", bufs=3) as p3w,
                tc.tile_pool(name=f"p3s_{s}", bufs=2) as p3s,
            ):
                fnorm = p3.tile([128, c.DT], F32, name="fnorm")
                nc.sync.dma_start(fnorm[:], ins[f"fnorm_{s}"])
                for icw in range(c.TC // c.TCW):
                    tw = c.TCW
                    wsl = slice(icw * tw, (icw + 1) * tw)
                    h_sb = p3.tile([128, c.DT, tw], F16, tag="h",
                                   name="h_sb")
                    nc.sync.dma_start(
                        h_sb[:], sc[f"h_{s}"][:, wsl].rearrange(
                            "(o p) t -> p o t", p=128))
                    zg = p3.tile([128, c.FT, tw], F16, tag="zg", name="zg")
                    for ft in range(c.FT):
                        w1 = p3w.tile([128, c.DT, 128], F16, tag="w1",
                                      name="w1")
                        nc.sync.dma_start(
                            w1[:],
                            ins[f"w1T_{s}"][:, ft * 128:(ft + 1) * 128]
                            .rearrange("(o p) j -> p o j", p=128))
                        w3 = p3w.tile([128, c.DT, 128], F16, tag="w3",
                                      name="w3")
                        nc.sync.dma_start(
                            w3[:],
                            ins[f"w3T_{s}"][:, ft * 128:(ft + 1) * 128]
                            .rearrange("(o p) j -> p o j", p=128))
                        z1 = mm([128, tw], "z1")
                        z3 = mm([128, tw], "z3")
                        for o in range(c.DT):
                            nc.tensor.matmul(z1[:], w1[:, o], h_sb[:, o],
                                             start=(o == 0),
                                             stop=(o == c.DT - 1))
                        for o in range(c.DT):
                            nc.tensor.matmul(z3[:], w3[:, o], h_sb[:, o],
                                             start=(o == 0),
                                             stop=(o == c.DT - 1))
                        sg = p3s.tile([128, tw], F16, tag="sg", name="sg")
                        nc.scalar.activation(sg[:], z1[:], AF.Sigmoid)
                        sl = p3s.tile([128, tw], F16, tag="sl", name="sl")
                        nc.vector.tensor_mul(sl[:], z1[:], sg[:])
                        nc.vector.tensor_mul(zg[:, ft], z3[:], sl[:])

                    r_all = p3.tile([128, c.DT, tw], F32, tag="r",
                                    name="r_all")
                    res = p3.tile([128, c.DT, tw], F32, tag="res",
                                  name="res")
                    nc.sync.dma_start(
                        res[:], ins[f"res_{s}"][:, wsl].rearrange(
                            "(o p) t -> p o t", p=128))
                    ns_ps = row([1, tw], "ns")
                    for dt in range(c.DT):
                        w2 = p3w.tile([128, c.FT, 128], F16, tag="w2",
                                      name="w2")
                        nc.sync.dma_start(
                            w2[:],
                            ins[f"w2T_{s}"][:, dt * 128:(dt + 1) * 128]
                            .rearrange("(o p) j -> p o j", p=128))
                        fp = mm([128, tw], "fp")
                        for ft in range(c.FT):
                            nc.tensor.matmul(fp[:], w2[:, ft], zg[:, ft],
                                             start=(ft == 0),
                                             stop=(ft == c.FT - 1))
                        nc.vector.tensor_add(r_all[:, dt], fp[:],
                                             res[:, dt])
                        r2 = p3s.tile([128, tw], F16, tag="r2", name="r2")
                        nc.vector.tensor_mul(r2[:], r_all[:, dt],
                                             r_all[:, dt])
                        nc.tensor.matmul(ns_ps[:], ones_col[:], r2[:],
                                         start=(dt == 0),
                                         stop=(dt == c.DT - 1))
                    rmsn = p3s.tile([1, tw], F32, tag="rmsn", name="rmsn")
                    nc.scalar.activation(rmsn[:], ns_ps[:], AF.Sqrt,
                                         bias=eps1[:], scale=one_over_d)
                    rsqn = p3s.tile([1, tw], F32, tag="rsqn", name="rsqn")
                    nc.vector.reciprocal(rsqn[:], rmsn[:])
                    rsqn16 = p3s.tile([1, tw], F16, tag="rsqn16",
                                      name="rsqn16")
                    nc.vector.tensor_copy(rsqn16[:], rsqn[:])
                    bcn = bcast_free(rsqn16, tw, p3s, f"fn{s}")
                    for dt in range(c.DT):
                        otl = p3s.tile([128, tw], F32, tag="otl", name="otl")
                        nc.vector.tensor_mul(otl[:], r_all[:, dt], bcn[:])
                        ofn = p3s.tile([128, tw], F32, tag="ofn", name="ofn")
                        nc.scalar.activation(ofn[:], otl[:], AF.Copy,
                                             scale=fnorm[:, dt:dt + 1])
                        nc.sync.dma_start(
                            outs[s][dt * 128:(dt + 1) * 128, wsl], ofn[:])


# ======================= host-side wrapper =========================

_CACHE = {}


def _prep_inputs(cfg, x, y, attn_norm_w,
                 wq_x, wk_x, wv_x, wo_x, wq_y, wk_y, wv_y, wo_y,
                 w1_x, w2_x, w3_x, ffn_norm_x,
                 w1_y, w2_y, w3_y, ffn_norm_y):
    c = cfg
    f16 = np.float16
    nw = np.asarray(attn_norm_w, np.float32)
    qscale = nw / np.sqrt(c.HD)

    def t16(a):
        return np.ascontiguousarray(np.asarray(a, np.float32).T).astype(f16)

    per_core = [dict() for _ in range(NCORES)]
    shared = {}
    for s, (xv, wq, wk, wv, wo, w1, w2, w3, fn) in {
        "x": (x, wq_x, wk_x, wv_x, wo_x, w1_x, w2_x, w3_x, ffn_norm_x),
        "y": (y, wq_y, wk_y, wv_y, wo_y, w1_y, w2_y, w3_y, ffn_norm_y),
    }.items():
        xt = np.asarray(xv, np.float32).reshape(c.T, c.D).T  # [D, T]
        shared[f"{s}T"] = np.ascontiguousarray(xt).astype(f16)
        wqT = (np.asarray(wq, np.float32) * qscale[None, :]).T  # [D, D]
        wkT = (np.asarray(wk, np.float32) * nw[None, :]).T
        wvT = (np.asarray(wv, np.float32) * nw[None, :]).T
        woT = np.asarray(wo, np.float32).T                     # [Din, Dout]
        shared[f"w1T_{s}"] = t16(w1)
        shared[f"w3T_{s}"] = t16(w3)
        shared[f"w2T_{s}"] = t16(w2)
        shared[f"fnorm_{s}"] = np.ascontiguousarray(
            np.asarray(fn, np.float32).reshape(c.DT, 128).T)
        for r in range(NCORES):
            js = slice(r * c.NQ, (r + 1) * c.NQ)
            ts = slice(r * c.TC, (r + 1) * c.TC)
            per_core[r][f"wqT_{s}"] = np.ascontiguousarray(wqT[:, js]).astype(f16)
            per_core[r][f"wkT_{s}"] = np.ascontiguousarray(wkT[:, js]).astype(f16)
            per_core[r][f"wvT_{s}"] = np.ascontiguousarray(wvT[:, js]).astype(f16)
            per_core[r][f"woT_{s}"] = np.ascontiguousarray(woT[js, :]).astype(f16)
            per_core[r][f"res_{s}"] = np.ascontiguousarray(xt[:, ts])
    in_maps = []
    for r in range(NCORES):
        m = dict(shared)
        m.update(per_core[r])
        in_maps.append(m)
    return in_maps


def run(cfg, inputs, **kw):
    from concourse import bass_utils

    key = (cfg.B, cfg.S, cfg.D, cfg.H, cfg.HD, cfg.FF)
    if key not in _CACHE:
        _CACHE[key] = build(cfg)
    nc = _CACHE[key]
    in_maps = _prep_inputs(cfg, **{k: v for k, v in inputs.items()
                                   if k != "start_pos"})
    res = bass_utils.run_bass_kernel_spmd(
        nc, in_maps, core_ids=list(range(NCORES)), **kw)
    outs = []
    for s in ("x", "y"):
        cols = [res.results[r][f"out_{s}"] for r in range(NCORES)]
        full_t = np.concatenate(cols, axis=1)           # [D, T]
        outs.append(np.ascontiguousarray(full_t.T)
                    .reshape(cfg.B, cfg.S, cfg.D).astype(np.float32))
    return tuple(outs), res


def kernel(**inputs):
    (out_x, out_y), _ = run(FULL, inputs)
    return out_x, out_y


# revision 7
# speedup vs baseline: 1.0837x; 1.0837x over previous
"""CrossAttentionBlockLLaMA on 8 Trainium2 NeuronCores (Bass/Tile).

Sharding:
  - QKV + attention: tensor-parallel over heads (2 heads/core).
  - Output projection wo: row-sharded over heads; each core computes a
    partial h for ALL tokens, written window-major [8, D, TC]; a
    ReduceScatter sums partials and hands core r exactly h.T[:, tokens_r].
  - FFN + post-norm: token-parallel (TC tokens/core), full weights.

Layouts: host pre-transposes activations/weights so every matmul's
contraction dim is on SBUF partitions. attn_norm_w and 1/sqrt(HD) are
folded into wq/wk/wv host-side; per-token 1/rms factors are applied to
q/k/v on device. Matmul inputs fp16 (validated ~6e-7 end-to-end rel err),
PSUM accumulation fp32, residual + final norm fp32.

Self-contained: hardcodes shapes from the problem spec.
"""
import numpy as np

NCORES = 8
EPS = 1e-5


class Cfg:
    def __init__(self, B=2, S=2048, D=2048, H=16, HD=128, FF=5632):
        self.B, self.S, self.D, self.H, self.HD, self.FF = B, S, D, H, HD, FF
        self.T = B * S                    # total tokens
        self.TC = self.T // NCORES        # tokens per core (phase 3)
        self.NQ = (H // NCORES) * HD      # per-core head dims
        self.DT = D // 128                # d-tiles
        self.FT = FF // 128               # ff-tiles
        self.NQT = self.NQ // 128         # per-core head-dim tiles
        self.TCH = min(512, self.T)       # phase-1 token chunk
        self.QCH = min(512, S)            # phase-2 query chunk
        self.TCW = min(512, self.TC)      # phase-3 / wo token chunk
        assert self.T % self.TCH == 0 and S % self.QCH == 0
        assert self.TC % self.TCW == 0 and S % 128 == 0
        assert HD == 128 and D % 128 == 0 and FF % 128 == 0


FULL = Cfg()


def build(cfg=FULL):
    import concourse.mybir as mybir
    import concourse.tile as tile
    from concourse import bacc

    F16 = mybir.dt.float16
    F32 = mybir.dt.float32

    c = cfg
    nc = bacc.Bacc("TRN2", target_bir_lowering=False, debug=False,
                   num_devices=NCORES)

    ins = {}
    outs = {}
    for s in ("x", "y"):
        ins[f"{s}T"] = nc.dram_tensor(f"{s}T", [c.D, c.T], F16,
                                      kind="ExternalInput").ap()
        for w in ("wq", "wk", "wv"):
            ins[f"{w}T_{s}"] = nc.dram_tensor(
                f"{w}T_{s}", [c.D, c.NQ], F16, kind="ExternalInput").ap()
        ins[f"woT_{s}"] = nc.dram_tensor(
            f"woT_{s}", [c.NQ, c.D], F16, kind="ExternalInput").ap()
        ins[f"w1T_{s}"] = nc.dram_tensor(
            f"w1T_{s}", [c.D, c.FF], F16, kind="ExternalInput").ap()
        ins[f"w3T_{s}"] = nc.dram_tensor(
            f"w3T_{s}", [c.D, c.FF], F16, kind="ExternalInput").ap()
        ins[f"w2T_{s}"] = nc.dram_tensor(
            f"w2T_{s}", [c.FF, c.D], F16, kind="ExternalInput").ap()
        ins[f"res_{s}"] = nc.dram_tensor(
            f"res_{s}", [c.D, c.TC], F32, kind="ExternalInput").ap()
        ins[f"fnorm_{s}"] = nc.dram_tensor(
            f"fnorm_{s}", [128, c.DT], F32, kind="ExternalInput").ap()
        outs[s] = nc.dram_tensor(f"out_{s}", [c.D, c.TC], F32,
                                 kind="ExternalOutput").ap()

    with tile.TileContext(nc) as tc:
        _emit(tc, nc, c, ins, outs)
    nc.compile()
    return nc


def _emit(tc, nc, c, ins, outs):
    import concourse.mybir as mybir

    F16 = mybir.dt.float16
    F32 = mybir.dt.float32
    AF = mybir.ActivationFunctionType
    one_over_d = 1.0 / c.D

    with (
        tc.tile_pool(name="psum", bufs=1, space="PSUM") as ps,
        tc.tile_pool(name="const", bufs=1) as const,
        tc.tile_pool(name="dram", bufs=1, space="DRAM") as dram,
    ):
        ones_col = const.tile([128, 1], F16)
        nc.vector.memset(ones_col[:], 1.0)
        ones_row = const.tile([1, 128], F16)
        nc.vector.memset(ones_row[:], 1.0)
        one11 = const.tile([1, 1], F32)
        nc.vector.memset(one11[:], 1.0)
        eps1 = const.tile([1, 1], F32)
        nc.vector.memset(eps1[:], EPS)

        sc = {}
        for s in ("x", "y"):
            sc[f"qT_{s}"] = dram.tile([c.NQ, c.T], F16, name=f"qT_{s}")
            sc[f"kT_{s}"] = dram.tile([c.NQ, c.T], F16, name=f"kT_{s}")
            sc[f"v_{s}"] = dram.tile([c.T, c.NQ], F16, name=f"v_{s}")
            sc[f"o_{s}"] = dram.tile([c.NQ, c.T], F16, name=f"o_{s}")
            # wo partials, window-major: [NCORES windows, D, TC]
            sc[f"hp_{s}"] = dram.tile([NCORES * c.D, c.TC], F16,
                                      name=f"hp_{s}")
            sc[f"h_{s}"] = dram.tile([c.D, c.TC], F16, name=f"h_{s}")

        def mm(shape, name):
            return ps.tile(shape, F32, tag="mm", bufs=6, name=name)

        def row(shape, name):
            return ps.tile(shape, F32, tag="row", bufs=2, name=name)

        def bcast_free(rsq16, width, sb_pool, name):
            """[1,width] f16 -> [128,width] f16 via DRAM stride-0 DMA.

            Keeps the broadcast entirely off the PE queue so the PE never
            stalls on the DVE reciprocal chain (HAM stays warm)."""
            rd = dram.tile([1, width], F16, tag="bc_row", bufs=4,
                           name=f"bcd_{name}")
            nc.sync.dma_start(rd[:], rsq16[:1, :width])
            bc16 = sb_pool.tile([128, width], F16, tag="bc16",
                                name=f"bc16_{name}")
            nc.sync.dma_start(bc16[:], rd[:].to_broadcast((128, width)))
            return bc16

        # ============ PHASE 1: RMSNorm stats + QKV projections =============
        with (
            tc.tile_pool(name="p1w", bufs=1) as p1w,
            tc.tile_pool(name="p1a", bufs=2) as p1a,
            tc.tile_pool(name="p1s", bufs=3) as p1s,
        ):
            W = {}
            for s in ("x", "y"):
                for w in ("wq", "wk", "wv"):
                    t = p1w.tile([128, c.DT, c.NQ], F16, name=f"{w}_{s}_sb")
                    nc.sync.dma_start(
                        t[:],
                        ins[f"{w}T_{s}"].rearrange("(o p) j -> p o j", p=128))
                    W[f"{w}{s}"] = t

            for ich in range(c.T // c.TCH):
                tsl = slice(ich * c.TCH, (ich + 1) * c.TCH)
                act = {}
                rsq_free = {}
                rsq_part = {}
                for s in ("x", "y"):
                    at = p1a.tile([128, c.DT, c.TCH], F16, tag=f"act_{s}",
                                  name=f"act_{s}")
                    nc.sync.dma_start(
                        at[:],
                        ins[f"{s}T"][:, tsl].rearrange("(o p) t -> p o t",
                                                       p=128))
                    act[s] = at

                    ms_ps = row([1, c.TCH], f"ms_{s}")
                    for o in range(c.DT):
                        sq = p1s.tile([128, c.TCH], F16, tag="sq",
                                      name=f"sq_{s}{o}")
                        nc.vector.tensor_mul(sq[:], at[:, o], at[:, o])
                        nc.tensor.matmul(ms_ps[:], ones_col[:], sq[:],
                                         start=(o == 0), stop=(o == c.DT - 1))
                    rms = p1s.tile([1, c.TCH], F32, tag="rms",
                                   name=f"rms_{s}")
                    nc.scalar.activation(rms[:], ms_ps[:], AF.Sqrt,
                                         bias=eps1[:], scale=one_over_d)
                    rsqf = p1s.tile([1, c.TCH], F32, tag="rsqf",
                                    name=f"rsqf_{s}")
                    nc.vector.reciprocal(rsqf[:], rms[:])
                    rsqf16 = p1s.tile([1, c.TCH], F16, tag="rsqf16",
                                      name=f"rsqf16_{s}")
                    nc.vector.tensor_copy(rsqf16[:], rsqf[:])
                    rsq_free[s] = rsqf16

                    nsub = c.TCH // 128
                    rfd = dram.tile([1, c.TCH], F32, tag="rsq_row", bufs=4,
                                    name=f"rfd_{s}")
                    nc.sync.dma_start(rfd[:], rsqf[:])
                    rsqT = p1s.tile([128, nsub], F32, tag="rsqT",
                                    name=f"rsqT_{s}")
                    nc.sync.dma_start(
                        rsqT[:], rfd[0, :].rearrange("(n p) -> p n", p=128))
                    rsq_part[s] = rsqT

                for s in ("x", "y"):
                    kv = "y" if s == "x" else "x"
                    bc_q = bcast_free(rsq_free[s], c.TCH, p1s, f"q{s}{ich}")
                    bc_k = bcast_free(rsq_free[kv], c.TCH, p1s, f"k{s}{ich}")

                    for (wname, src, bc, dst) in (
                        ("wq", s, bc_q, sc[f"qT_{s}"]),
                        ("wk", kv, bc_k, sc[f"kT_{s}"]),
                    ):
                        for jt in range(c.NQT):
                            pm = mm([128, c.TCH], f"{wname}{s}{jt}")
                            wt = W[f"{wname}{s}"]
                            for o in range(c.DT):
                                nc.tensor.matmul(
                                    pm[:], wt[:, o, jt * 128:(jt + 1) * 128],
                                    act[src][:, o],
                                    start=(o == 0), stop=(o == c.DT - 1))
                            ot = p1s.tile([128, c.TCH], F16, tag="proj_out",
                                          name=f"{wname}{s}{jt}o")
                            nc.vector.tensor_mul(ot[:], pm[:], bc[:])
                            nc.sync.dma_start(
                                dst[jt * 128:(jt + 1) * 128, tsl], ot[:])

                    for i in range(c.TCH // 128):
                        pv = mm([128, c.NQ], f"v{s}{i}")
                        for o in range(c.DT):
                            nc.tensor.matmul(
                                pv[:], act[kv][:, o, i * 128:(i + 1) * 128],
                                W[f"wv{s}"][:, o, :],
                                start=(o == 0), stop=(o == c.DT - 1))
                        vt = p1s.tile([128, c.NQ], F16, tag="v_out",
                                      name=f"v{s}{i}o")
                        nc.vector.tensor_scalar_mul(
                            vt[:], pv[:], rsq_part[kv][:, i:i + 1])
                        nc.sync.dma_start(
                            sc[f"v_{s}"][ich * c.TCH + i * 128:
                                         ich * c.TCH + (i + 1) * 128, :],
                            vt[:])

        # ============ PHASE 2: attention + wo partial + ReduceScatter ======
        for s in ("x", "y"):
            with tc.tile_pool(name=f"p2_{s}", bufs=2) as p2:
                for b in range(c.B):
                    bsl = slice(b * c.S, (b + 1) * c.S)
                    for h in range(c.NQT):
                        hsl = slice(h * 128, (h + 1) * 128)
                        kt = p2.tile([128, c.S], F16, tag="kt", name="kt")
                        nc.sync.dma_start(kt[:], sc[f"kT_{s}"][hsl, bsl])
                        vt = p2.tile([128, c.S // 128, 128], F16, tag="vt",
                                     name="vt")
                        nc.sync.dma_start(
                            vt[:], sc[f"v_{s}"][bsl, hsl].rearrange(
                                "(n p) j -> p n j", p=128))
                        for q0 in range(0, c.S, c.QCH):
                            qsl = slice(b * c.S + q0, b * c.S + q0 + c.QCH)
                            qt = p2.tile([128, c.QCH], F16, tag="qt",
                                         name="qt")
                            nc.sync.dma_start(qt[:], sc[f"qT_{s}"][hsl, qsl])
                            o_ps = mm([128, c.QCH], "o_ps")
                            sum_ps = row([1, c.QCH], "sum_ps")
                            nk = c.S // 128
                            for ik in range(nk):
                                s_ps = mm([128, c.QCH], "s_ps")
                                nc.tensor.matmul(
                                    s_ps[:], kt[:, ik * 128:(ik + 1) * 128],
                                    qt[:], start=True, stop=True)
                                e16 = p2.tile([128, c.QCH], F16, tag="e16",
                                              bufs=6, name="e16")
                                nc.scalar.activation(e16[:], s_ps[:], AF.Exp)
                                nc.tensor.matmul(sum_ps[:], ones_col[:],
                                                 e16[:], start=(ik == 0),
                                                 stop=(ik == nk - 1))
                                nc.tensor.matmul(o_ps[:], vt[:, ik], e16[:],
                                                 start=(ik == 0),
                                                 stop=(ik == nk - 1))
                            rs_ = p2.tile([1, c.QCH], F32, tag="rs",
                                          name="rs")
                            nc.vector.reciprocal(rs_[:], sum_ps[:])
                            rs16 = p2.tile([1, c.QCH], F16, tag="rs16",
                                           name="rs16")
                            nc.vector.tensor_copy(rs16[:], rs_[:])
                            bc16 = bcast_free(rs16, c.QCH, p2, "at")
                            on16 = p2.tile([128, c.QCH], F16, tag="on16",
                                           name="on16")
                            nc.vector.tensor_mul(on16[:], o_ps[:], bc16[:])
                            nc.sync.dma_start(sc[f"o_{s}"][hsl, qsl],
                                              on16[:])

                # ---- wo partial for ALL tokens, written window-major ----
                wo_sb = p2.tile([128, c.NQT, c.D], F16, tag="wo", bufs=1,
                                name="wo_sb")
                nc.sync.dma_start(
                    wo_sb[:],
                    ins[f"woT_{s}"].rearrange("(o p) j -> p o j", p=128))
                for w in range(NCORES):
                    for u in range(c.TC // c.TCW):
                        t0 = w * c.TC + u * c.TCW
                        ot = p2.tile([128, c.NQT, c.TCW], F16, tag="ot",
                                     name="ot")
                        nc.sync.dma_start(
                            ot[:], sc[f"o_{s}"][:, t0:t0 + c.TCW].rearrange(
                                "(o p) t -> p o t", p=128))
                        for dt in range(c.DT):
                            hp = mm([128, c.TCW], "hp")
                            for o in range(c.NQT):
                                nc.tensor.matmul(
                                    hp[:],
                                    wo_sb[:, o, dt * 128:(dt + 1) * 128],
                                    ot[:, o], start=(o == 0),
                                    stop=(o == c.NQT - 1))
                            hp16 = p2.tile([128, c.TCW], F16, tag="hp16",
                                           name="hp16")
                            nc.scalar.activation(hp16[:], hp[:], AF.Copy)
                            nc.sync.dma_start(
                                sc[f"hp_{s}"][w * c.D + dt * 128:
                                              w * c.D + (dt + 1) * 128,
                                              u * c.TCW:(u + 1) * c.TCW],
                                hp16[:])

            nc.gpsimd.collective_compute(
                "ReduceScatter", mybir.AluOpType.add,
                replica_groups=[list(range(NCORES))],
                ins=[sc[f"hp_{s}"][:].opt()],
                outs=[sc[f"h_{s}"][:].opt()],
            )

        # ============ PHASE 3: SwiGLU FFN + residual + post-norm ===========
        for s in ("x", "y"):
            with (
                tc.tile_pool(name=f"p3_{s}", bufs=1) as p3,
                tc.tile_pool(name=f"p3w_{s}", bufs=5) as p3w,
                tc.tile_pool(name=f"p3s_{s}", bufs=2) as p3s,
            ):
                fnorm = p3.tile([128, c.DT], F32, name="fnorm")
                nc.sync.dma_start(fnorm[:], ins[f"fnorm_{s}"])
                for icw in range(c.TC // c.TCW):
                    tw = c.TCW
                    wsl = slice(icw * tw, (icw + 1) * tw)
                    h_sb = p3.tile([128, c.DT, tw], F16, tag="h",
                                   name="h_sb")
                    nc.sync.dma_start(
                        h_sb[:], sc[f"h_{s}"][:, wsl].rearrange(
                            "(o p) t -> p o t", p=128))
                    zg = p3.tile([128, c.FT, tw], F16, tag="zg", name="zg")
                    for ft in range(c.FT):
                        w1 = p3w.tile([128, c.DT, 128], F16, tag="w1",
                                      name="w1")
                        nc.sync.dma_start(
                            w1[:],
                            ins[f"w1T_{s}"][:, ft * 128:(ft + 1) * 128]
                            .rearrange("(o p) j -> p o j", p=128))
                        w3 = p3w.tile([128, c.DT, 128], F16, tag="w3",
                                      name="w3")
                        nc.sync.dma_start(
                            w3[:],
                            ins[f"w3T_{s}"][:, ft * 128:(ft + 1) * 128]
                            .rearrange("(o p) j -> p o j", p=128))
                        z1 = mm([128, tw], "z1")
                        z3 = mm([128, tw], "z3")
                        for o in range(c.DT):
                            nc.tensor.matmul(z1[:], w1[:, o], h_sb[:, o],
                                             start=(o == 0),
                                             stop=(o == c.DT - 1))
                        for o in range(c.DT):
                            nc.tensor.matmul(z3[:], w3[:, o], h_sb[:, o],
                                             start=(o == 0),
                                             stop=(o == c.DT - 1))
                        sg = p3s.tile([128, tw], F16, tag="sg", name="sg")
                        nc.scalar.activation(sg[:], z1[:], AF.Sigmoid)
                        sl = p3s.tile([128, tw], F16, tag="sl", name="sl")
                        nc.vector.tensor_mul(sl[:], z1[:], sg[:])
                        nc.vector.tensor_mul(zg[:, ft], z3[:], sl[:])

                    r_all = p3.tile([128, c.DT, tw], F32, tag="r",
                                    name="r_all")
                    ns_ps = row([1, tw], "ns")
                    for dt in range(c.DT):
                        w2 = p3w.tile([128, c.FT, 128], F16, tag="w2", bufs=3,
                                      name="w2")
                        nc.sync.dma_start(
                            w2[:],
                            ins[f"w2T_{s}"][:, dt * 128:(dt + 1) * 128]
                            .rearrange("(o p) j -> p o j", p=128))
                        fp = mm([128, tw], "fp")
                        for ft in range(c.FT):
                            nc.tensor.matmul(fp[:], w2[:, ft], zg[:, ft],
                                             start=(ft == 0),
                                             stop=(ft == c.FT - 1))
                        res = p3s.tile([128, tw], F32, tag="res", bufs=3,
                                       name="res")
                        nc.sync.dma_start(
                            res[:],
                            ins[f"res_{s}"][dt * 128:(dt + 1) * 128, wsl])
                        nc.vector.tensor_add(r_all[:, dt], fp[:], res[:])
                        r2 = p3s.tile([128, tw], F16, tag="r2", name="r2")
                        nc.vector.tensor_mul(r2[:], r_all[:, dt],
                                             r_all[:, dt])
                        nc.tensor.matmul(ns_ps[:], ones_col[:], r2[:],
                                         start=(dt == 0),
                                         stop=(dt == c.DT - 1))
                    rmsn = p3s.tile([1, tw], F32, tag="rmsn", name="rmsn")
                    nc.scalar.activation(rmsn[:], ns_ps[:], AF.Sqrt,
                                         bias=eps1[:], scale=one_over_d)
                    rsqn = p3s.tile([1, tw], F32, tag="rsqn", name="rsqn")
                    nc.vector.reciprocal(rsqn[:], rmsn[:])
                    rsqn16 = p3s.tile([1, tw], F16, tag="rsqn16",
                                      name="rsqn16")
                    nc.vector.tensor_copy(rsqn16[:], rsqn[:])
                    bcn = bcast_free(rsqn16, tw, p3s, f"fn{s}")
                    for dt in range(c.DT):
                        otl = p3s.tile([128, tw], F32, tag="otl", name="otl")
                        nc.vector.tensor_mul(otl[:], r_all[:, dt], bcn[:])
                        ofn = p3s.tile([128, tw], F32, tag="ofn", name="ofn")
                        nc.scalar.activation(ofn[:], otl[:], AF.Copy,
                                             scale=fnorm[:, dt:dt + 1])
                        nc.sync.dma_start(
                            outs[s][dt * 128:(dt + 1) * 128, wsl], ofn[:])


# ======================= host-side wrapper =========================

_CACHE = {}


def _prep_inputs(cfg, x, y, attn_norm_w,
                 wq_x, wk_x, wv_x, wo_x, wq_y, wk_y, wv_y, wo_y,
                 w1_x, w2_x, w3_x, ffn_norm_x,
                 w1_y, w2_y, w3_y, ffn_norm_y):
    c = cfg
    f16 = np.float16
    nw = np.asarray(attn_norm_w, np.float32)
    qscale = nw / np.sqrt(c.HD)

    def t16(a):
        return np.ascontiguousarray(np.asarray(a, np.float32).T).astype(f16)

    per_core = [dict() for _ in range(NCORES)]
    shared = {}
    for s, (xv, wq, wk, wv, wo, w1, w2, w3, fn) in {
        "x": (x, wq_x, wk_x, wv_x, wo_x, w1_x, w2_x, w3_x, ffn_norm_x),
        "y": (y, wq_y, wk_y, wv_y, wo_y, w1_y, w2_y, w3_y, ffn_norm_y),
    }.items():
        xt = np.asarray(xv, np.float32).reshape(c.T, c.D).T  # [D, T]
        shared[f"{s}T"] = np.ascontiguousarray(xt).astype(f16)
        wqT = (np.asarray(wq, np.float32) * qscale[None, :]).T  # [D, D]
        wkT = (np.asarray(wk, np.float32) * nw[None, :]).T
        wvT = (np.asarray(wv, np.float32) * nw[None, :]).T
        woT = np.asarray(wo, np.float32).T                     # [Din, Dout]
        shared[f"w1T_{s}"] = t16(w1)
        shared[f"w3T_{s}"] = t16(w3)
        shared[f"w2T_{s}"] = t16(w2)
        shared[f"fnorm_{s}"] = np.ascontiguousarray(
            np.asarray(fn, np.float32).reshape(c.DT, 128).T)
        for r in range(NCORES):
            js = slice(r * c.NQ, (r + 1) * c.NQ)
            ts = slice(r * c.TC, (r + 1) * c.TC)
            per_core[r][f"wqT_{s}"] = np.ascontiguousarray(wqT[:, js]).astype(f16)
            per_core[r][f"wkT_{s}"] = np.ascontiguousarray(wkT[:, js]).astype(f16)
            per_core[r][f"wvT_{s}"] = np.ascontiguousarray(wvT[:, js]).astype(f16)
            per_core[r][f"woT_{s}"] = np.ascontiguousarray(woT[js, :]).astype(f16)
            per_core[r][f"res_{s}"] = np.ascontiguousarray(xt[:, ts])
    in_maps = []
    for r in range(NCORES):
        m = dict(shared)
        m.update(per_core[r])
        in_maps.append(m)
    return in_maps


def run(cfg, inputs, **kw):
    from concourse import bass_utils

    key = (cfg.B, cfg.S, cfg.D, cfg.H, cfg.HD, cfg.FF)
    if key not in _CACHE:
        _CACHE[key] = build(cfg)
    nc = _CACHE[key]
    in_maps = _prep_inputs(cfg, **{k: v for k, v in inputs.items()
                                   if k != "start_pos"})
    res = bass_utils.run_bass_kernel_spmd(
        nc, in_maps, core_ids=list(range(NCORES)), **kw)
    outs = []
    for s in ("x", "y"):
        cols = [res.results[r][f"out_{s}"] for r in range(NCORES)]
        full_t = np.concatenate(cols, axis=1)           # [D, T]
        outs.append(np.ascontiguousarray(full_t.T)
                    .reshape(cfg.B, cfg.S, cfg.D).astype(np.float32))
    return tuple(outs), res


def kernel(**inputs):
    (out_x, out_y), _ = run(FULL, inputs)
    return out_x, out_y
